# revision 2
# baseline (speedup 1.0000x reference)
"""MinCutNet (2x GCN + dense_mincut_pool losses) as an 8-core Trainium2
Bass/Tile kernel — v5: dense-adjacency matmuls + raw RDMA exchanges.

Structure (per core, dst-sharded):
  - The normalized GCN adjacency (with self loops) is built densely on the
    host: each core streams its [10240 src, 1280 dst] fp8 block T1 once and
    keeps it resident in SBUF.  T1[src, dst] = A_gcn[dst, src].
  - L1: aggT1[f, dst] = x^T @ T1 (fp8 DoubleRow), then y1 = relu(agg@W1+b1)
    per dst block — no PE transposes needed anywhere.
  - y1 (fp8) is allgathered SBUF->SBUF with remote_dma_broadcast rounds
    (relative XOR addressing; no collectives).  Slot d holds the shard of
    the physical XOR-d neighbour; the host permutes T1's src-block order to
    match, so arrival order is compile-time known.
  - L2: aggT2[f2, dst] = y1_all^T @ T1, pipelined against the exchange
    (slot-s matmuls wait on the cumulative arrival semaphore).
  - Dense W2/Wp + softmax per dst block (transposed dataflow as in the
    RS-based kernel, minus the transposes).
  - The mincut numerator uses tr(s^T A_raw s) = u^T A_gcn u - tr(s^T s)
    with u = s/dis, so the pool phase reuses the SAME resident T1 block:
    v = A_gcn@u needs one tiny u allgather (fp8) instead of a second
    adjacency stream + ReduceScatter.
  - Final scalars (ss partial, num/den/trss partials) ride one last tiny
    exchange; every core reduces them and computes the loss (core 0's
    output is returned).

Sem-count races: data broadcasts carry the arrival semaphore (inc 16 for
same-die rounds, 8 for cross-die); receive buffers are never written
locally (no memset — a fast peer's write would race it).
"""

import os
import sys

sys.path.insert(0, "/opt/trn_rl_repo")

import numpy as np

# --- libnrt fallbacks: identity topology when no driver is present (sim).
# _TRN2_NC_BASE is the trn2 logical->physical NC map; verified on HW via the
# slot layout of a broadcast probe.
import concourse.libnrt as libnrt

NCB = (0, 1, 2, 3, 6, 7, 4, 5)


def _wrap(fn, fallback):
    def inner():
        try:
            return fn()
        except Exception:
            return fallback

    return inner


libnrt.get_device_id_to_routing_id_mapping = _wrap(
    libnrt.get_device_id_to_routing_id_mapping, {i: i for i in range(16)}
)
libnrt.get_trn2_nc_mapping = _wrap(
    libnrt.get_trn2_nc_mapping,
    {(d, i): NCB[i] for d in range(16) for i in range(8)},
)
import concourse.bass_interp as bass_interp

bass_interp.get_device_id_to_routing_id_mapping = (
    libnrt.get_device_id_to_routing_id_mapping
)

import concourse.bass as bass
import concourse.mybir as mybir
import concourse.tile as tile
from concourse import library_config
from concourse.bass_utils import run_bass_kernel_spmd
from concourse.library_overlay import lower_extended_insts

import ml_dtypes

# ---------------------------------------------------------------- constants
N, E = 10000, 320000
FIN, FH, K = 128, 256, 64
C = 8
P = 128
NPAD = 10240
SHARD = NPAD // C          # 1280
BLK = SHARD // P           # 10 dst blocks per core
NBLK = NPAD // P           # 80 src blocks
F32 = mybir.dt.float32
BF16 = mybir.dt.bfloat16
FP8 = mybir.dt.float8e4
NPFP8 = ml_dtypes.float8_e4m3
NPBF16 = ml_dtypes.bfloat16

# per-round arrival-sem increments (same-die rounds 1-3: 8 slots x 2 lanes;
# cross-die rounds 4-7: 4 slots x 2 lanes)
XINC = [16, 16, 16, 8, 8, 8, 8]
XTOT = sum(XINC)

_MAX_PHASE = int(os.environ.get("KERNEL_MAX_PHASE", "9"))
_DEBUG_OUTPUTS = bool(int(os.environ.get("KERNEL_DEBUG_OUTPUTS", "0")))


# ------------------------------------------------------- sync-info patching
def _patched_drain_and_barrier(self, tick_clock, wait_clock):
    """walrus in this container rejects >1 sync-wait command on the tail
    Drain; spread the waits across SP nops (1 wait each)."""
    from concourse.vector_clock import ScopedClock

    nc = self.nc
    drain_inst = nc.sync.drain()
    wait_clock.add_sem_waits(
        drain_inst.ins, ScopedClock({None: tick_clock.global_clock})
    )
    waits = list(drain_inst.ins.sync_info.on_wait)
    if len(waits) > 1:
        upd = list(drain_inst.ins.sync_info.on_update)
        drain_inst.ins.sync_info = mybir.SyncInfo(on_wait=waits[:1], on_update=upd)
        for i, w in enumerate(waits[1:]):
            nop = nc.sync.nop(nofuse=True, hint=f"tailwait{i}")
            nop.ins.sync_info = mybir.SyncInfo(on_wait=[w], on_update=[])
    nc.all_engine_barrier()
    assert self.sems is not None
    popped = nc._tile_sem_poison_stack.pop()
    assert popped is self._sem_poison
    nc.clear_and_free_semaphores(list(self.sems.allocated().values()))
    nc.all_engine_barrier()


tile.TileContext._drain_and_barrier = _patched_drain_and_barrier

_noop_ctr = [0]


def _split_excess_waits(nc, lim=1):
    """walrus caps sync-wait commands per instruction; spill excess waits
    onto same-engine NOPs placed just before."""
    for fn in nc.m.functions:
        for b in fn.blocks:
            newl = []
            changed = False
            for inst in b.instructions:
                si = inst.sync_info
                if si is not None and len(si.on_wait) > lim:
                    waits = list(si.on_wait)
                    head, tail = waits[: len(waits) - lim], waits[len(waits) - lim :]
                    for i in range(0, len(head), lim):
                        _noop_ctr[0] += 1
                        nop = mybir.InstNoOp(
                            name=f"waitnop-{_noop_ctr[0]}",
                            sync_info=mybir.SyncInfo(
                                on_wait=head[i : i + lim], on_update=[]
                            ),
                            bass_nofuse=True,
                            engine=inst.engine,
                        )
                        newl.append(nop)
                    inst.sync_info = mybir.SyncInfo(
                        on_wait=tail, on_update=list(si.on_update)
                    )
                    changed = True
                newl.append(inst)
            if changed:
                b.instructions = newl


def _attach_waits(pending, sems):
    """Attach sem-ge waits recorded as (BassInstruction, (round, value)) AFTER
    the tile scheduling pass (the single-core scheduling sim cannot observe
    cross-core sem arrivals and would deadlock on them)."""
    for bi, (rnd, value) in pending:
        sem = sems[rnd - 1]
        w = mybir.SyncWait(
            sync_type="semaphore", id=sem.num, ant_name=sem.name,
            wait_mode="sem-ge-imm", wait_value=value,
        )
        si = bi.ins.sync_info
        if si is None:
            bi.ins.sync_info = mybir.SyncInfo(on_wait=[w], on_update=[])
        else:
            bi.ins.sync_info = mybir.SyncInfo(
                on_wait=list(si.on_wait) + [w], on_update=list(si.on_update)
            )


# ------------------------------------------------------- host preprocessing
def _slot_owner(core, s):
    """logical core whose shard lands in slot s on `core` (XOR of physical
    ids; NCB is self-inverse)."""
    return NCB[NCB[core] ^ s]


def preprocess(edge_index, edge_weight):
    row = edge_index[0].astype(np.int64)
    col = edge_index[1].astype(np.int64)
    ew = edge_weight.astype(np.float32)

    # gcn_norm on host: deg over col with self loops
    deg = np.zeros(NPAD, np.float32)
    np.add.at(deg, col, ew)
    deg[:N] += 1.0
    deg[deg == 0] = 1.0
    dis = (1.0 / np.sqrt(deg)).astype(np.float32)

    # dense normalized adjacency A[dst, src] (+ self loops)
    A = np.zeros((NPAD, NPAD), np.float32)
    np.add.at(A, (col, row), dis[row] * ew * dis[col])
    A[np.arange(N), np.arange(N)] += dis[:N] ** 2

    # d = raw out-degree sums (adj.sum(-1) in the reference)
    d = np.zeros(NPAD, np.float32)
    np.add.at(d, row, ew)

    mask = np.zeros(NPAD, np.float32)
    mask[:N] = 1.0
    rdis = np.sqrt(deg).astype(np.float32)  # 1/dis

    t1 = []
    orders = []
    for c in range(C):
        order = np.concatenate(
            [
                np.arange(SHARD, dtype=np.int64) + _slot_owner(c, s) * SHARD
                for s in range(C)
            ]
        )
        orders.append(order)
        # T1[p, b, dcol] = A[c*SHARD + dcol, order[b*128 + p]]
        blockT = np.ascontiguousarray(
            A[c * SHARD : (c + 1) * SHARD, order].T
        ).astype(NPFP8)
        t1.append(blockT.reshape(NBLK, P, SHARD).transpose(1, 0, 2))

    def per_core_dst(v):  # [NPAD] -> [C][P, BLK] with v[c*1280 + b*128 + p]
        return np.ascontiguousarray(
            v.reshape(C, BLK, P).transpose(0, 2, 1)
        )

    return dict(
        t1=np.ascontiguousarray(np.stack(t1)).reshape(C, P, NBLK * SHARD),
        orders=np.stack(orders),
        d=per_core_dst(d),
        mask=per_core_dst(mask),
        rdis=per_core_dst(rdis),
    )


# --------------------------------------------------------- device program
def build_program():
    nc = bass.Bass(num_devices=C, detect_race_conditions=False)
    dp = nc.declare_dram_parameter

    t1_t = dp("t1", [P, NBLK * SHARD], FP8, isOutput=False)
    xall_t = dp("xall", [P, NBLK * FIN], FP8, isOutput=False)
    w1 = dp("W1", [FIN, FH], BF16, isOutput=False)
    w2 = dp("W2", [P, 2, 2, P], BF16, isOutput=False)   # [fin_p, fc, oc, fout_p]
    wp = dp("Wp", [P, 2, K], BF16, isOutput=False)      # [fout_p, oc, k]
    b1 = dp("b1", [1, FH], F32, isOutput=False)
    b2r = dp("b2r", [1, 2, P], F32, isOutput=False)
    bp = dp("bp", [1, K], F32, isOutput=False)
    d_t = dp("d", [P, BLK], F32, isOutput=False)
    mask_t = dp("mask", [P, BLK], F32, isOutput=False)
    rdis_t = dp("rdis", [P, BLK], F32, isOutput=False)
    ones_t = dp("ones", [P, 1], F32, isOutput=False)
    ones_row_t = dp("ones_row", [1, P], F32, isOutput=False)
    out_t = dp("out", [1, 1], F32, isOutput=True)

    dbg = {}
    if _DEBUG_OUTPUTS:
        dbg["y1"] = dp("dbg_y1", [P, BLK * FH], F32, isOutput=True)
        dbg["s"] = dp("dbg_s", [P, BLK * K], F32, isOutput=True)
        dbg["v"] = dp("dbg_v", [P, BLK * K], F32, isOutput=True)
        dbg["yall"] = dp("dbg_yall", [P, NBLK * FH], FP8, isOutput=True)
        dbg["agg2"] = dp("dbg_agg2", [P, 2 * SHARD], BF16, isOutput=True)
        dbg["fin"] = dp("dbg_fin", [P, 68], F32, isOutput=True)

    nc.gpsimd.load_library(library_config.remote_dma)

    xsems = [nc.alloc_semaphore(f"xch_arrive{d}") for d in range(1, 8)]
    rsem = nc.alloc_semaphore("rendezvous")
    lsem = nc.alloc_semaphore("xch_local")

    pending_waits = []   # (BassInstruction, (round, threshold))
    pending_rendezvous = []
    xch_round = [0]      # number of completed exchanges

    def exchange(in_ap, out_slot_ap):
        """7 broadcast rounds: slot d of the receive buffer gets the shard of
        the XOR-d physical neighbour. Round d bumps its OWN semaphore (a
        single counter is unsound: arrivals from 7 different senders
        interleave, so a cumulative count can be reached before a given
        round's data has landed). Returns per-round (round, threshold)."""
        xch_round[0] += 1
        k = xch_round[0]
        for dlt in range(1, 8):
            rdests = [(0, dlt)] * 8 if dlt < 4 else [None] * 4 + [(0, dlt)] * 4
            nc.gpsimd.remote_dma_broadcast(
                out_ap=out_slot_ap(dlt),
                in_ap=in_ap,
                remote_sem=xsems[dlt - 1],
                local_sem=lsem,
                rdests=rdests,
            )
        trig = nc.gpsimd.trigger_dma(count=None)
        if k == 1:
            pending_rendezvous.append(trig)
        return [None] + [(dlt, k * XINC[dlt - 1]) for dlt in range(1, 8)]

    with tile.TileContext(nc) as tc:
        with (
            tc.tile_pool(name="const", bufs=1) as cp,
            tc.tile_pool(name="big", bufs=1) as bigp,       # T1 + exchange bufs
            tc.tile_pool(name="work", bufs=2) as wk,
            tc.tile_pool(name="acc", bufs=1) as accp,
            tc.tile_pool(name="ps", bufs=1, space="PSUM") as ps,
            tc.tile_pool(name="psd", bufs=1, space="PSUM") as psd,
        ):
            # ---------------- constants
            def load(pool, name, src, shape, dtype=F32, eng=None):
                t = pool.tile(shape, dtype, tag=name)
                (eng or nc.scalar).dma_start(out=t[:], in_=src)
                return t

            w1_sb = load(cp, "w1", w1[:], [P, FH], BF16)
            w2_sb = load(cp, "w2", w2[:], [P, 2, 2, P], BF16)
            wp_sb = load(cp, "wp", wp[:], [P, 2, K], BF16)
            b1_sb = load(cp, "b1", b1[:], [1, FH])
            b2r_sb = load(cp, "b2r", b2r[:], [1, 2, P])
            bp_sb = load(cp, "bp", bp[:], [1, K])
            d_sb = load(cp, "d", d_t[:], [P, BLK])
            mask_sb = load(cp, "mask", mask_t[:], [P, BLK])
            rdis_sb = load(cp, "rdis", rdis_t[:], [P, BLK])
            ones_sb = load(cp, "ones", ones_t[:], [P, 1])
            ones_row_sb = load(cp, "ones_row", ones_row_t[:], [1, P])
            xall_sb = load(
                cp, "xall", xall_t[:].rearrange("p (b f) -> p b f", f=FIN),
                [P, NBLK, FIN], FP8,
            )

            # ---------------- exchange / resident buffers (never written
            # locally outside their own slot 0)
            t1_sb = bigp.tile([P, NBLK, SHARD], FP8, tag="t1")
            y1_all = bigp.tile([P, NBLK, FH], FP8, tag="y1all")
            u_all = bigp.tile([P, NBLK, K], FP8, tag="uall")
            fin_recv = bigp.tile([P, 8, 68], F32, tag="finrecv")

            # ---------------- rendezvous: sem-only wave proving every core
            # is executing before any data rdma flies (guards dispatch skew)
            for dlt in range(1, 8):
                rdests = [(0, dlt)] * 8 if dlt < 4 else [None] * 4 + [(0, dlt)] * 4
                nc.gpsimd.remote_sem_update_broadcast(
                    remote_sem=rsem, local_sem=lsem, rdests=rdests,
                )
            nc.gpsimd.trigger_dma(count=None)

            # ---------------- T1 stream (8 chunks of 10 src blocks)
            t1_dr = t1_t[:].rearrange("p (b d) -> p b d", d=SHARD)
            NCHUNK = 8
            CB = NBLK // NCHUNK
            for ci in range(NCHUNK):
                nc.sync.dma_start(
                    out=t1_sb[:, ci * CB : (ci + 1) * CB, :],
                    in_=t1_dr[:, ci * CB : (ci + 1) * CB, :],
                )

            # ---------------- L1: aggT1[f, dst] = x^T @ T1
            DCH = [(0, 512), (512, 1024), (1024, 1280)]
            pA = [
                ps.tile([P, 2, 512], F32, tag=f"q{i}", name=f"pq{i}")[:, 0, 0 : c1 - c0]
                for i, (c0, c1) in enumerate(DCH)
            ]
            if _MAX_PHASE >= 1:
                for ci in range(NCHUNK):
                    for pp in range(CB // 2):
                        sb = ci * CB + 2 * pp
                        for i, (c0, c1) in enumerate(DCH):
                            nc.tensor.matmul(
                                pA[i],
                                xall_sb[:, sb : sb + 2, :],
                                t1_sb[:, sb : sb + 2, c0:c1],
                                start=(ci == 0 and pp == 0),
                                stop=(ci == NCHUNK - 1 and pp == CB // 2 - 1),
                                perf_mode=mybir.MatmulPerfMode.DoubleRow,
                            )
                aggT1 = wk.tile([P, SHARD], BF16, tag="aggT1")
                nc.vector.tensor_copy(aggT1[:, 0:512], pA[0])
                nc.vector.tensor_copy(aggT1[:, 512:1024], pA[1])
                nc.scalar.copy(aggT1[:, 1024:1280], pA[2])

                # y1 = relu(agg @ W1 + b1) per dst block, straight into slot 0
                for b in range(BLK):
                    h_ps = psd.tile([P, FH], F32, tag="mm")
                    nc.tensor.matmul(
                        h_ps[:], aggT1[:, b * P : (b + 1) * P], w1_sb[:],
                        start=True, stop=False,
                    )
                    nc.tensor.matmul(
                        h_ps[:], ones_row_sb[:], b1_sb[:], start=False, stop=True
                    )
                    nc.scalar.activation(
                        y1_all[:, b, :], h_ps[:], mybir.ActivationFunctionType.Relu
                    )
                if _DEBUG_OUTPUTS:
                    dy = wk.tile([P, BLK, FH], F32, tag="dy")
                    for b in range(BLK):
                        nc.vector.tensor_copy(dy[:, b, :], y1_all[:, b, :])
                    nc.sync.dma_start(
                        out=dbg["y1"][:].rearrange("p (b f) -> p b f", f=FH),
                        in_=dy[:],
                    )

            # ---------------- y1 exchange
            if _MAX_PHASE >= 2:
                cum_y1 = exchange(
                    y1_all[:, 0:BLK, :], lambda d: y1_all[:, d * BLK : (d + 1) * BLK, :]
                )

            # ---------------- L2: aggT2[f2, dst] = y1_all^T @ T1
            if _MAX_PHASE >= 3:
                pB = [
                    ps.tile([P, 2, 512], F32, tag=f"q{i}", name=f"pq{i}")
                    for i, (c0, c1) in enumerate(DCH)
                ]
                for s in range(8):
                    first = True
                    for pp in range(BLK // 2):
                        sb = s * BLK + 2 * pp
                        for i, (c0, c1) in enumerate(DCH):
                            for f2c in range(2):
                                mm = nc.tensor.matmul(
                                    pB[i][:, f2c, 0 : c1 - c0],
                                    y1_all[:, sb : sb + 2, f2c * P : (f2c + 1) * P],
                                    t1_sb[:, sb : sb + 2, c0:c1],
                                    start=(s == 0 and pp == 0),
                                    stop=(s == 7 and pp == BLK // 2 - 1),
                                    perf_mode=mybir.MatmulPerfMode.DoubleRow,
                                )
                                if s > 0 and first:
                                    pending_waits.append((mm, cum_y1[s]))
                                    first = False
                aggT2 = wk.tile([P, 2, SHARD], BF16, tag="aggT2")
                for i, (c0, c1) in enumerate(DCH):
                    nc.vector.tensor_copy(aggT2[:, 0, c0:c1], pB[i][:, 0, 0 : c1 - c0])
                    nc.scalar.copy(aggT2[:, 1, c0:c1], pB[i][:, 1, 0 : c1 - c0])
                if _DEBUG_OUTPUTS:
                    ya = nc.sync.dma_start(
                        out=dbg["yall"][:].rearrange("p (b f) -> p b f", f=FH),
                        in_=y1_all[:],
                    )
                    pending_waits.append((ya, cum_y1[7]))
                    nc.sync.dma_start(
                        out=dbg["agg2"][:].rearrange("p (a d) -> p a d", d=SHARD),
                        in_=aggT2[:],
                    )

            # ---------------- dense W2/Wp + softmax + u per dst block
            s_sb = accp.tile([P, BLK, K], F32, tag="s")
            ssq_sb = accp.tile([P, BLK], F32, tag="ssq")
            accn_sb = accp.tile([P, BLK], F32, tag="accn")
            sscratch = wk.tile([P, K], F32, tag="sscratch")
            if _MAX_PHASE >= 4:
                for b in range(BLK):
                    h2t_ps = psd.tile([P, 2, P], F32, tag="mm")
                    for oc in range(2):
                        nc.tensor.matmul(
                            h2t_ps[:, oc, :], b2r_sb[:, oc, :], ones_row_sb[:],
                            start=True, stop=False,
                        )
                        for fc in range(2):
                            nc.tensor.matmul(
                                h2t_ps[:, oc, :],
                                w2_sb[:, fc, oc, :],
                                aggT2[:, fc, b * P : (b + 1) * P],
                                start=False, stop=(fc == 1),
                            )
                    o2t = wk.tile([P, 2, P], BF16, tag="o2t")
                    nc.vector.tensor_scalar_max(o2t[:], h2t_ps[:], 0.0)
                    sp = psd.tile([P, K], F32, tag="sp")
                    for oc in range(2):
                        nc.tensor.matmul(
                            sp[:], o2t[:, oc, :], wp_sb[:, oc, :],
                            start=(oc == 0), stop=False,
                        )
                    nc.tensor.matmul(
                        sp[:], ones_row_sb[:], bp_sb[:], start=False, stop=True
                    )
                    smax = wk.tile([P, 1], F32, tag="smax")
                    nc.vector.tensor_reduce(
                        smax[:], sp[:], axis=mybir.AxisListType.X,
                        op=mybir.AluOpType.max, negate=True,
                    )
                    sexp = wk.tile([P, K], F32, tag="sexp")
                    ssum = wk.tile([P, 1], F32, tag="ssum")
                    nc.scalar.activation(
                        sexp[:], sp[:], mybir.ActivationFunctionType.Exp,
                        bias=smax[:], accum_out=ssum[:],
                    )
                    nc.vector.reciprocal(ssum[:], ssum[:])
                    nc.vector.tensor_scalar(
                        s_sb[:, b, :], sexp[:], ssum[:], mask_sb[:, b : b + 1],
                        op0=mybir.AluOpType.mult, op1=mybir.AluOpType.mult,
                    )
                    nc.vector.tensor_tensor(
                        out=sscratch[:], in0=s_sb[:, b, :], in1=s_sb[:, b, :],
                        op=mybir.AluOpType.mult,
                    )
                    nc.vector.tensor_reduce(
                        ssq_sb[:, b : b + 1], sscratch[:],
                        axis=mybir.AxisListType.X, op=mybir.AluOpType.add,
                    )
                    # u = s * sqrt(deg), fp8 into slot 0 of the exchange buffer
                    nc.vector.tensor_scalar(
                        u_all[:, b, :], s_sb[:, b, :], rdis_sb[:, b : b + 1],
                        None, op0=mybir.AluOpType.mult,
                    )
                if _DEBUG_OUTPUTS:
                    nc.sync.dma_start(
                        out=dbg["s"][:].rearrange("p (b k) -> p b k", k=K),
                        in_=s_sb[:],
                    )

            # ---------------- u exchange + v = A_gcn @ u (resident T1)
            if _MAX_PHASE >= 5:
                cum_u = exchange(
                    u_all[:, 0:BLK, :], lambda d: u_all[:, d * BLK : (d + 1) * BLK, :]
                )
                v_sb = accp.tile([P, BLK, K], F32, tag="v_sb")
                vq = [
                    ps.tile([P, 2, 512], F32, tag=f"q{i}", name=f"pq{i}")
                    for i in range(2)
                ]
                for g0 in range(0, BLK, 4):
                    blocks = list(range(g0, min(g0 + 4, BLK)))
                    for s in range(8):
                        first = True
                        for pp in range(BLK // 2):
                            sb = s * BLK + 2 * pp
                            for bi_, b in enumerate(blocks):
                                mm = nc.tensor.matmul(
                                    vq[bi_ // 2][:, bi_ % 2, 0:K],
                                    t1_sb[:, sb : sb + 2, b * P : (b + 1) * P],
                                    u_all[:, sb : sb + 2, :],
                                    start=(s == 0 and pp == 0),
                                    stop=(s == 7 and pp == BLK // 2 - 1),
                                    perf_mode=mybir.MatmulPerfMode.DoubleRow,
                                )
                                if g0 == 0 and s > 0 and first:
                                    pending_waits.append((mm, cum_u[s]))
                                    first = False
                    for bi_, b in enumerate(blocks):
                        if bi_ % 2 == 0:
                            nc.vector.tensor_copy(
                                v_sb[:, b, :], vq[bi_ // 2][:, bi_ % 2, 0:K]
                            )
                        else:
                            nc.scalar.copy(
                                v_sb[:, b, :], vq[bi_ // 2][:, bi_ % 2, 0:K]
                            )
                # num partial: sum_dst,k u_own * v
                vdbg = wk.tile([P, BLK, K], F32, tag="vdbg", name="vdbg") if _DEBUG_OUTPUTS else None
                for b in range(BLK):
                    t = wk.tile([P, K], F32, tag="uv")
                    nc.vector.tensor_tensor(
                        out=t[:], in0=s_sb[:, b, :], in1=v_sb[:, b, :],
                        op=mybir.AluOpType.mult,
                    )
                    if _DEBUG_OUTPUTS:
                        nc.scalar.copy(vdbg[:, b, :], v_sb[:, b, :])
                    red = wk.tile([P, 1], F32, tag="uvred")
                    nc.vector.tensor_reduce(
                        red[:], t[:], axis=mybir.AxisListType.X,
                        op=mybir.AluOpType.add,
                    )
                    # u = s*rdis -> scale the reduced row by rdis
                    nc.vector.tensor_scalar(
                        accn_sb[:, b : b + 1], red[:], rdis_sb[:, b : b + 1],
                        None, op0=mybir.AluOpType.mult,
                    )
                if _DEBUG_OUTPUTS:
                    nc.sync.dma_start(
                        out=dbg["v"][:].rearrange("p (b k) -> p b k", k=K),
                        in_=vdbg[:],
                    )

            # ---------------- finale partials + tiny exchange + loss
            if _MAX_PHASE >= 6:
                fin_ps = psd.tile([P, 68], F32, tag="mm", name="fin_ps")
                ss_ps = fin_ps[0:K, 0:K]
                for b in range(BLK):
                    nc.tensor.matmul(
                        ss_ps, s_sb[:, b, :], s_sb[:, b, :],
                        start=(b == 0), stop=(b == BLK - 1),
                    )
                # num partial -> [0,64]
                redn = wk.tile([P, 1], F32, tag="redn")
                nc.vector.tensor_reduce(
                    redn[:], accn_sb[:], axis=mybir.AxisListType.X,
                    op=mybir.AluOpType.add,
                )
                nc.tensor.matmul(
                    fin_ps[0:1, 64:65], redn[:], ones_sb[:], start=True, stop=True
                )
                # den partial -> [0,65]
                den_sb = wk.tile([P, BLK], F32, tag="den")
                nc.vector.tensor_tensor(
                    out=den_sb[:], in0=ssq_sb[:], in1=d_sb[:],
                    op=mybir.AluOpType.mult,
                )
                red2 = wk.tile([P, 1], F32, tag="red2")
                nc.vector.tensor_reduce(
                    red2[:], den_sb[:], axis=mybir.AxisListType.X,
                    op=mybir.AluOpType.add,
                )
                nc.tensor.matmul(
                    fin_ps[0:1, 65:66], red2[:], ones_sb[:], start=True, stop=True
                )
                # trss partial -> [0,66]
                red3 = wk.tile([P, 1], F32, tag="red3")
                nc.vector.tensor_reduce(
                    red3[:], ssq_sb[:], axis=mybir.AxisListType.X,
                    op=mybir.AluOpType.add,
                )
                nc.tensor.matmul(
                    fin_ps[0:1, 66:67], red3[:], ones_sb[:], start=True, stop=True
                )
                fin_send = wk.tile([P, 68], F32, tag="finsend")
                nc.vector.memset(fin_send[:], 0.0)
                nc.vector.tensor_copy(fin_send[0:K, 0:K], ss_ps)
                nc.vector.tensor_copy(fin_send[0:1, 64:67], fin_ps[0:1, 64:67])
                nc.vector.tensor_copy(fin_recv[:, 0, :], fin_send[:])
                cum_f = exchange(fin_send[:], lambda d: fin_recv[:, d, :])

                acc = wk.tile([P, 68], F32, tag="facc")
                cop = nc.vector.tensor_copy(acc[:], fin_recv[:, 0, :])
                for j in range(1, 8):
                    ad = nc.vector.tensor_tensor(
                        out=acc[:], in0=acc[:], in1=fin_recv[:, j, :],
                        op=mybir.AluOpType.add,
                    )
                    pending_waits.append((ad, cum_f[j]))
                if _DEBUG_OUTPUTS:
                    nc.sync.dma_start(out=dbg["fin"][:], in_=acc[:])

                # loss = ortho - (num_u - trss)/den
                # ortho = sqrt(2 - 2*trss/(fro*sqrt(K)))
                sq64 = wk.tile([K, K], F32, tag="sq64")
                col64 = wk.tile([K, 1], F32, tag="col64")
                nc.vector.tensor_tensor(
                    out=sq64[:], in0=acc[0:K, 0:K], in1=acc[0:K, 0:K],
                    op=mybir.AluOpType.mult,
                )
                nc.vector.tensor_reduce(
                    col64[:], sq64[:], axis=mybir.AxisListType.X,
                    op=mybir.AluOpType.add,
                )
                fro_ps = psd.tile([P, 1], F32, tag="sp", name="fro_ps")
                nc.tensor.matmul(
                    fro_ps[0:1, 0:1], col64[:], ones_sb[0:K, :], start=True, stop=True
                )
                rfro = wk.tile([1, 1], F32, tag="rfro")
                nc.scalar.sqrt(rfro[:], fro_ps[0:1, 0:1])
                nc.vector.reciprocal(rfro[:], rfro[:])
                o2 = wk.tile([1, 1], F32, tag="o2s")
                nc.vector.tensor_tensor(
                    out=o2[:], in0=acc[0:1, 66:67], in1=rfro[:],
                    op=mybir.AluOpType.mult,
                )
                nc.vector.tensor_scalar(
                    o2[:], o2[:], -2.0 / float(np.sqrt(K)), 2.0,
                    op0=mybir.AluOpType.mult, op1=mybir.AluOpType.add,
                )
                orth = wk.tile([1, 1], F32, tag="orth")
                nc.scalar.sqrt(orth[:], o2[:])
                numt = wk.tile([1, 1], F32, tag="numt")
                nc.vector.tensor_tensor(
                    out=numt[:], in0=acc[0:1, 64:65], in1=acc[0:1, 66:67],
                    op=mybir.AluOpType.subtract,
                )
                rden = wk.tile([1, 1], F32, tag="rden")
                nc.vector.reciprocal(rden[:], acc[0:1, 65:66])
                mloss = wk.tile([1, 1], F32, tag="mloss")
                nc.vector.tensor_tensor(
                    out=mloss[:], in0=numt[:], in1=rden[:],
                    op=mybir.AluOpType.mult,
                )
                res = wk.tile([1, 1], F32, tag="res")
                nc.vector.tensor_tensor(
                    out=res[:], in0=orth[:], in1=mloss[:],
                    op=mybir.AluOpType.subtract,
                )
                nc.sync.dma_start(out=out_t[:], in_=res[:])
            else:
                zz = wk.tile([1, 1], F32, tag="zz")
                nc.vector.memset(zz[:], 0.0)
                nc.sync.dma_start(out=out_t[:], in_=zz[:])

    _attach_waits(pending_waits, xsems)
    for bi in pending_rendezvous:
        w = mybir.SyncWait(
            sync_type="semaphore", id=rsem.num, ant_name=rsem.name,
            wait_mode="sem-ge-imm", wait_value=XTOT,
        )
        si = bi.ins.sync_info
        bi.ins.sync_info = mybir.SyncInfo(
            on_wait=(list(si.on_wait) if si else []) + [w],
            on_update=(list(si.on_update) if si else []),
        )
    nc.clear_and_free_semaphores(xsems + [rsem, lsem])
    _split_excess_waits(nc)
    lower_extended_insts(nc)
    return nc


_PROG = [None]


def _get_program():
    if _PROG[0] is None:
        _PROG[0] = build_program()
    return _PROG[0]


def make_in_maps(inputs, tabs):
    x = np.asarray(inputs["x"], np.float32)
    W1, W2, Wp = (np.asarray(inputs[k], np.float32) for k in ("W1", "W2", "Wp"))
    b1, b2, bp = (np.asarray(inputs[k], np.float32) for k in ("b1", "b2", "bp"))
    xpad = np.zeros((NPAD, FIN), np.float32)
    xpad[:N] = x
    xpad8 = xpad.astype(NPFP8)

    w2r = np.ascontiguousarray(
        W2.reshape(2, P, 2, P).transpose(1, 0, 2, 3)
    ).astype(NPBF16)
    wpr = np.ascontiguousarray(Wp.reshape(2, P, K).transpose(1, 0, 2)).astype(NPBF16)
    b2rr = np.ascontiguousarray(b2.reshape(1, 2, P)).astype(np.float32)

    common = dict(
        W1=W1.astype(NPBF16),
        W2=w2r,
        Wp=wpr,
        b1=b1.reshape(1, FH).astype(np.float32),
        b2r=b2rr,
        bp=bp.reshape(1, K).astype(np.float32),
        ones=np.ones((P, 1), np.float32),
        ones_row=np.ones((1, P), np.float32),
    )
    in_maps = []
    for c in range(C):
        order = tabs["orders"][c]
        xc = np.ascontiguousarray(
            xpad8[order].reshape(NBLK, P, FIN).transpose(1, 0, 2)
        ).reshape(P, NBLK * FIN)
        in_maps.append(
            dict(
                common,
                t1=tabs["t1"][c],
                xall=xc,
                d=tabs["d"][c],
                mask=tabs["mask"][c],
                rdis=tabs["rdis"][c],
            )
        )
    return in_maps


def kernel(x, edge_index, edge_weight, W1, b1, W2, b2, Wp, bp):
    edge_index = np.asarray(edge_index)
    edge_weight = np.asarray(edge_weight, np.float32)
    tabs = preprocess(edge_index, edge_weight)
    nc = _get_program()
    in_maps = make_in_maps(
        dict(x=x, W1=W1, b1=b1, W2=W2, b2=b2, Wp=Wp, bp=bp), tabs
    )
    last_err = None
    for _attempt in range(3):
        try:
            res = run_bass_kernel_spmd(nc, in_maps, core_ids=list(range(C)))
            break
        except Exception as e:  # transient device/dispatch flakes: retry
            last_err = e
    else:
        raise last_err
    out = res.results[0]["out"].reshape(())
    if _DEBUG_OUTPUTS:
        kernel.debug = {
            k: [res.results[c].get(f"dbg_{k}") for c in range(C)]
            for k in ("y1", "s", "v", "fin", "yall", "agg2")
        }
    return np.float32(out)


if __name__ == "__main__":
    import reference

    inputs = reference.setup_inputs()
    inputs = {k: np.asarray(v) for k, v in inputs.items()}
    got = kernel(**inputs)
    print("kernel out:", got)
